# revision 15
# baseline (speedup 1.0000x reference)
"""MinimalGPT forward on 8 Trainium2 NeuronCores.

Sharding: sequence-parallel transformer + vocab-parallel head.
  - core c: batch b = c//4, group rank r = c%4, owns sequence chunks (r, 7-r)
    of its batch element (2 x 128 = 256 tokens), zigzag for causal balance.
  - activations flow TRANSPOSED: xT [D, tok] so weights load as natural lhsT.
  - per layer: local qT/kT/vT projections (V transposed to natural via PE);
    AllGather of (kT, v) within the 4-core batch group; attention over
    gathered keys with a host-built causal mask (uniform SPMD program,
    per-core mask data); local Wo/LN/FFN/LN.
  - final: LN, AllGather hT across all 8 cores, each core computes a 4096-col
    vocab slice of the logits; host reassembles + transposes.

All matmuls run as float32r (full-rate PE, fp32 accumulate in PSUM).
"""

import math
import numpy as np
from contextlib import ExitStack

import concourse.bass as bass
import concourse.tile as tile
from concourse import bacc, mybir
from concourse.bass_utils import run_bass_kernel_spmd
from concourse.masks import make_identity

f32 = mybir.dt.float32
f32r = mybir.dt.float32r
i32 = mybir.dt.int32
AF = mybir.ActivationFunctionType
OP = mybir.AluOpType

V, D, H, L, F = 32000, 768, 12, 6, 3072
B, S = 2, 1024
P = 128
DK = 64
DCH = D // P           # 6
FCH = F // P           # 24
TOK = 256              # tokens per core
NCORE, GRP = 8, 4
VPAD = 4096            # padded vocab shard per core
VCH = VPAD // P        # 32
EPS = 1e-5
SQD = math.sqrt(D)
ISQDK = 1.0 / math.sqrt(DK)





def build(nc):
    # ---------------- DRAM I/O ----------------
    def din(name, shape, dt=f32):
        return nc.dram_tensor(name, shape, dt, kind="ExternalInput").ap()

    tok = din("tok", [P, 2], i32)
    emb = din("emb", [V, D])
    peT = din("peT", [D, TOK])
    maskt = din("maskt", [8 * P, TOK])
    wq = din("wq", [L, D, D], f32r)
    wk = din("wk", [L, D, D], f32r)
    wv = din("wv", [L, D, D], f32r)
    wo = din("wo", [L, D, D], f32r)
    w1 = din("w1", [L, D, F], f32r)
    w2 = din("w2", [L, F, D], f32r)
    bqp = din("bqp", [L, P, DCH])
    bkp = din("bkp", [L, P, DCH])
    bvp = din("bvp", [L, P, DCH])
    bop = din("bop", [L, P, DCH])
    b1p = din("b1p", [L, P, FCH])
    b2p = din("b2p", [L, P, DCH])
    g1p = din("g1p", [L, P, DCH])
    be1p = din("be1p", [L, P, DCH])
    g2p = din("g2p", [L, P, DCH])
    be2p = din("be2p", [L, P, DCH])
    gfp = din("gfp", [P, DCH])
    bfp = din("bfp", [P, DCH])
    woutc = din("woutc", [D, VPAD], f32r)
    boutp = din("boutp", [P, VCH])

    out = nc.dram_tensor("out", [VPAD, NCORE * TOK], f32, kind="ExternalOutput").ap()

    # internal DRAM bounce buffers (per layer to keep dependencies simple)
    kvins = [nc.dram_tensor(f"kvin{l}", [2 * D * TOK], f32r).ap() for l in range(L)]
    kvouts = [
        nc.dram_tensor(f"kvout{l}", [GRP * 2 * D * TOK], f32r).ap()
        for l in range(L)
    ]
    hin = nc.dram_tensor("hin", [D * TOK], f32r).ap()
    hout = nc.dram_tensor("hout", [NCORE * D * TOK], f32r, addr_space="Shared").ap()

    KV_GROUPS = [[0, 1, 2, 3], [4, 5, 6, 7]]
    ALL_GROUP = [list(range(NCORE))]

    with tile.TileContext(nc) as tc, ExitStack() as octx, \
            nc.allow_low_precision(reason="fp32r matmul inputs, fp32 accumulate"):
        const = octx.enter_context(tc.tile_pool(name="const", bufs=1))
        acts = octx.enter_context(tc.tile_pool(name="acts", bufs=2))
        stats = octx.enter_context(tc.tile_pool(name="stats", bufs=6))
        psum = octx.enter_context(tc.tile_pool(name="psum", bufs=8, space="PSUM"))

        def ctile(shape, dt, nm):
            return const.tile(shape, dt, name=nm, tag=nm)

        ident = ctile([P, P], f32, "ident")
        make_identity(nc, ident[:])
        ones_col_f = ctile([P, 1], f32, "ones_col_f")
        nc.vector.memset(ones_col_f[:], 1.0)
        ones_col = ctile([P, 1], f32r, "ones_col")
        nc.vector.tensor_copy(ones_col[:], ones_col_f[:])
        ones_row_f = ctile([1, P], f32, "ones_row_f")
        nc.vector.memset(ones_row_f[:], 1.0)
        ones_row = ctile([1, P], f32r, "ones_row")
        nc.vector.tensor_copy(ones_row[:], ones_row_f[:])
        eps_t = ctile([1, 1], f32, "eps_t")
        nc.vector.memset(eps_t[:], EPS)

        tokt = ctile([P, 2], i32, "tokt")
        nc.sync.dma_start(tokt[:], tok[:])

        mtiles = []
        for kb in range(8):
            mt = ctile([P, TOK], f32, f"mask{kb}")
            nc.sync.dma_start(mt[:], maskt[kb * P : (kb + 1) * P, :])
            mtiles.append(mt)

        gft = ctile([P, DCH], f32, "gft")
        nc.sync.dma_start(gft[:], gfp[:])
        bft = ctile([P, DCH], f32, "bft")
        nc.sync.dma_start(bft[:], bfp[:])
        boutt = ctile([P, VCH], f32, "boutt")
        nc.sync.dma_start(boutt[:], boutp[:])

        # ---------------- embedding ----------------
        x = [acts.tile([P, TOK], f32r, name=f"x{d}", tag="x", bufs=20)
             for d in range(DCH)]
        with ExitStack() as ectx:
            epool = ectx.enter_context(tc.tile_pool(name="epool", bufs=2))
            for j in range(2):
                g = epool.tile([P, D], f32, name="embrow", tag="embrow", bufs=2)
                nc.gpsimd.indirect_dma_start(
                    out=g[:],
                    out_offset=None,
                    in_=emb[:],
                    in_offset=bass.IndirectOffsetOnAxis(ap=tokt[:, j : j + 1], axis=0),
                )
                for dd in range(DCH):
                    pt = epool.tile([P, TOK], f32, name="pe", tag="pe", bufs=3)
                    nc.sync.dma_start(
                        pt[:, 0:P], peT[dd * P : (dd + 1) * P, j * P : (j + 1) * P]
                    )
                    ps = psum.tile([P, P], f32, name="tpose", tag="ps")
                    nc.tensor.transpose(ps[:], g[:, dd * P : (dd + 1) * P], ident[:])
                    sl = x[dd][:, j * P : (j + 1) * P]
                    nc.vector.tensor_scalar_mul(sl, ps[:], SQD)
                    nc.vector.tensor_add(sl, sl, pt[:, 0:P])

        # ---------------- helpers ----------------
        def linear_T(in_tiles, w_dram, l, wpool, nin, nout, wname):
            """outT[oc] psum [P,TOK] = sum_ic W[icP:+P, ocP:+P].T @ in[ic]."""
            outs = [psum.tile([P, TOK], f32, name=f"{wname}ps{oc}", tag="ps")
                    for oc in range(nout)]
            for ic in range(nin):
                wt = wpool.tile([P, nout * P], f32r, name=f"{wname}w",
                                tag="w", bufs=6)
                nc.sync.dma_start(wt[:], w_dram[l, ic * P : (ic + 1) * P, :])
                for oc in range(nout):
                    nc.tensor.matmul(
                        outs[oc][:],
                        lhsT=wt[:, oc * P : (oc + 1) * P],
                        rhs=in_tiles[ic][:],
                        start=(ic == 0),
                        stop=(ic == nin - 1),
                    )
            return outs

        def ln_T(in_tiles, g_ap, be_ap, outname, outtag="x", outbufs=20):
            """LayerNorm over the feature (=partition) dim of transposed tiles."""
            st_sum = psum.tile([1, TOK], f32, name="lnsum", tag="ps")
            for dd in range(DCH):
                nc.tensor.matmul(
                    st_sum[:], lhsT=ones_col[:], rhs=in_tiles[dd][:],
                    start=(dd == 0), stop=(dd == DCH - 1),
                )
            sqs = []
            for dd in range(DCH):
                sq = acts.tile([P, TOK], f32r, name="lnsq", tag="sq", bufs=4)
                nc.vector.tensor_mul(sq[:], in_tiles[dd][:], in_tiles[dd][:])
                sqs.append(sq)
            st_sq = psum.tile([1, TOK], f32, name="lnsq2", tag="ps")
            for dd in range(DCH):
                nc.tensor.matmul(
                    st_sq[:], lhsT=ones_col[:], rhs=sqs[dd][:],
                    start=(dd == 0), stop=(dd == DCH - 1),
                )
            nm = stats.tile([1, TOK], f32r, name="nm", tag="st")
            nc.vector.tensor_scalar_mul(nm[:], st_sum[:], -1.0 / D)
            ex2 = stats.tile([1, TOK], f32, name="ex2", tag="st")
            nc.vector.tensor_scalar_mul(ex2[:], st_sq[:], 1.0 / D)
            m2 = stats.tile([1, TOK], f32, name="m2", tag="st")
            nc.vector.tensor_mul(m2[:], nm[:], nm[:])
            var = stats.tile([1, TOK], f32, name="var", tag="st")
            nc.vector.tensor_sub(var[:], ex2[:], m2[:])
            std = stats.tile([1, TOK], f32, name="std", tag="st")
            nc.scalar.activation(std[:], var[:], AF.Sqrt, bias=eps_t[:])
            rstd = stats.tile([1, TOK], f32r, name="rstd", tag="st")
            nc.vector.reciprocal(rstd[:], std[:])
            nmb = psum.tile([P, TOK], f32, name="nmb", tag="ps")
            nc.tensor.matmul(nmb[:], lhsT=ones_row[:], rhs=nm[:],
                             start=True, stop=True)
            rsb = psum.tile([P, TOK], f32, name="rsb", tag="ps")
            nc.tensor.matmul(rsb[:], lhsT=ones_row[:], rhs=rstd[:],
                             start=True, stop=True)
            outs = []
            for dd in range(DCH):
                o = acts.tile([P, TOK], f32r, name=f"{outname}{dd}",
                              tag=outtag, bufs=outbufs)
                nc.vector.tensor_add(o[:], in_tiles[dd][:], nmb[:])
                nc.vector.tensor_mul(o[:], o[:], rsb[:])
                nc.vector.tensor_scalar(
                    o[:], o[:], g_ap[:, dd : dd + 1], be_ap[:, dd : dd + 1],
                    op0=OP.mult, op1=OP.add,
                )
                outs.append(o)
            return outs

        # ---------------- transformer layers ----------------
        with ExitStack() as lctx:
            wpool = lctx.enter_context(tc.tile_pool(name="wpool", bufs=6))
            w1pool = lctx.enter_context(tc.tile_pool(name="w1pool", bufs=7))
            bpool = lctx.enter_context(tc.tile_pool(name="bpool", bufs=22))
            ktall = lctx.enter_context(tc.tile_pool(name="ktall", bufs=24))
            vall = lctx.enter_context(tc.tile_pool(name="vall", bufs=11))
            expp = lctx.enter_context(tc.tile_pool(name="expp", bufs=4))
            ffp = lctx.enter_context(tc.tile_pool(name="ffp", bufs=25))

            for l in range(L):
                def bload(src, nm):
                    t = bpool.tile([P, src.shape[2]], f32, name=nm, tag="b",
                                   bufs=22)
                    nc.sync.dma_start(t[:], src[l])
                    return t

                bqt = bload(bqp, "bqt")
                bkt = bload(bkp, "bkt")
                bvt = bload(bvp, "bvt")
                bot = bload(bop, "bot")
                b2t = bload(b2p, "b2t")
                g1t = bload(g1p, "g1t")
                be1t = bload(be1p, "be1t")
                g2t = bload(g2p, "g2t")
                be2t = bload(be2p, "be2t")
                b1t = bload(b1p, "b1t")

                # ---- qT, kT, vT projections ----
                qps = linear_T(x, wq, l, wpool, DCH, DCH, "q")
                qT = []
                for oc in range(DCH):
                    t = acts.tile([P, TOK], f32r, name=f"qT{oc}", tag="qk", bufs=12)
                    nc.vector.tensor_scalar_add(t[:], qps[oc][:], bqt[:, oc : oc + 1])
                    qT.append(t)
                kps = linear_T(x, wk, l, wpool, DCH, DCH, "k")
                kT = []
                for oc in range(DCH):
                    t = acts.tile([P, TOK], f32r, name=f"kT{oc}", tag="qk", bufs=12)
                    nc.vector.tensor_scalar_add(t[:], kps[oc][:], bkt[:, oc : oc + 1])
                    kT.append(t)
                vps = linear_T(x, wv, l, wpool, DCH, DCH, "v")
                vT = []
                for oc in range(DCH):
                    t = acts.tile([P, TOK], f32, name=f"vT{oc}", tag="vt", bufs=7)
                    nc.vector.tensor_scalar_add(t[:], vps[oc][:], bvt[:, oc : oc + 1])
                    vT.append(t)
                # transpose vT -> natural V [tok, feat]
                vnat = []
                for tch in range(2):
                    vt = vall.tile([P, D], f32r, name=f"vnat{tch}", tag="v", bufs=11)
                    for dd in range(DCH):
                        ps = psum.tile([P, P], f32, name="vtp", tag="ps")
                        nc.tensor.transpose(
                            ps[:], vT[dd][:, tch * P : (tch + 1) * P], ident[:]
                        )
                        nc.vector.tensor_copy(vt[:, dd * P : (dd + 1) * P], ps[:])
                    vnat.append(vt)

                # ---- bounce out + AllGather(kT, v) within batch group ----
                kvin, kvout = kvins[l], kvouts[l]
                kin = kvin[0 : D * TOK].rearrange("(d p t) -> d p t", p=P, t=TOK)
                vin = kvin[D * TOK :].rearrange("(c p d) -> c p d", p=P, d=D)
                for dd in range(DCH):
                    nc.sync.dma_start(kin[dd], kT[dd][:])
                for tch in range(2):
                    nc.sync.dma_start(vin[tch], vnat[tch][:])
                nc.gpsimd.collective_compute(
                    "AllGather",
                    OP.bypass,
                    replica_groups=KV_GROUPS,
                    ins=[kvin.opt()],
                    outs=[kvout.opt()],
                )
                KTg = {}
                Vg = {}
                for g in range(GRP):
                    base = g * 2 * D * TOK
                    kg = kvout[base : base + D * TOK].rearrange(
                        "(d p t) -> d p t", p=P, t=TOK
                    )
                    vg = kvout[base + D * TOK : base + 2 * D * TOK].rearrange(
                        "(c p d) -> c p d", p=P, d=D
                    )
                    for dd in range(DCH):
                        t = ktall.tile([P, TOK], f32r, name=f"KT{g}_{dd}",
                                       tag="kt", bufs=24)
                        nc.sync.dma_start(t[:], kg[dd])
                        KTg[(g, dd)] = t
                    for ch in range(2):
                        t = vall.tile([P, D], f32r, name=f"Vg{g}_{ch}",
                                      tag="v", bufs=11)
                        nc.sync.dma_start(t[:], vg[ch])
                        Vg[(g, ch)] = t

                # ---- attention ----
                oT = []
                for hp in range(DCH):
                    oTs = []
                    dens = []
                    for sub in range(2):
                        h = 2 * hp + sub
                        prow = sub * DK
                        oT_ps = psum.tile([DK, TOK], f32, name="oTps", tag="ps")
                        oTs.append(oT_ps)
                        den = psum.tile([1, TOK], f32, name=f"den{sub}", tag="ps")
                        dens.append(den)
                        for kb in range(8):
                            g, ch = divmod(kb, 2)
                            sc = psum.tile([P, TOK], f32, name="scps", tag="ps")
                            nc.tensor.matmul(
                                sc[:],
                                lhsT=KTg[(g, hp)][prow : prow + DK,
                                                    ch * P : (ch + 1) * P],
                                rhs=qT[hp][prow : prow + DK, :],
                                start=True,
                                stop=True,
                            )
                            e = expp.tile([P, TOK], f32r, name="exptile",
                                          tag="e", bufs=4)
                            nc.vector.tensor_add(e[:], sc[:], mtiles[kb][:])
                            nc.scalar.activation(e[:], e[:], AF.Exp, scale=ISQDK)
                            nc.tensor.matmul(
                                oT_ps[:],
                                lhsT=Vg[(g, ch)][:, h * DK : (h + 1) * DK],
                                rhs=e[:],
                                start=(kb == 0),
                                stop=(kb == 7),
                                skip_group_check=True,
                            )
                            nc.tensor.matmul(
                                den[:],
                                lhsT=ones_col[:],
                                rhs=e[:],
                                start=(kb == 0),
                                stop=(kb == 7),
                            )
                    for sub in range(2):
                        rec = stats.tile([1, TOK], f32r, name="rec", tag="st")
                        nc.vector.reciprocal(rec[:], dens[sub][:])
                        rb = psum.tile([DK, TOK], f32, name="rbps", tag="ps")
                        nc.tensor.matmul(
                            rb[:], lhsT=ones_row[:, 0:DK], rhs=rec[:],
                            start=True, stop=True, skip_group_check=True,
                        )
                        rbs = acts.tile([DK, TOK], f32, name="rbs", tag="rbs",
                                        bufs=3)
                        nc.vector.tensor_copy(rbs[:], rb[:])
                        ot = acts.tile([DK, TOK], f32r, name=f"oT{hp}_{sub}",
                                       tag="oT", bufs=14)
                        nc.vector.tensor_mul(ot[:], oTs[sub][:], rbs[:])
                        oT.append(ot)

                # ---- output projection + residual + LN1 ----
                ops_ = [psum.tile([P, TOK], f32, name=f"ops{oc}", tag="ps")
                        for oc in range(DCH)]
                for h in range(H):
                    wt = wpool.tile([DK, D], f32r, name="wow", tag="w", bufs=6)
                    nc.sync.dma_start(wt[:], wo[l, h * DK : (h + 1) * DK, :])
                    for oc in range(DCH):
                        nc.tensor.matmul(
                            ops_[oc][:],
                            lhsT=wt[:, oc * P : (oc + 1) * P],
                            rhs=oT[h][:],
                            start=(h == 0),
                            stop=(h == H - 1),
                        )
                t1 = []
                for oc in range(DCH):
                    t = acts.tile([P, TOK], f32r, name=f"t1_{oc}", tag="x", bufs=20)
                    nc.vector.tensor_scalar_add(t[:], ops_[oc][:], bot[:, oc : oc + 1])
                    nc.vector.tensor_add(t[:], t[:], x[oc][:])
                    t1.append(t)
                xn1 = ln_T(t1, g1t, be1t, "xn1_")

                # ---- FFN ----
                ffs = []
                for og in range(4):
                    w1ts = []
                    for ic in range(DCH):
                        wt = w1pool.tile([P, DCH * P], f32r, name="w1t",
                                         tag="w1", bufs=7)
                        nc.sync.dma_start(
                            wt[:],
                            w1[l, ic * P : (ic + 1) * P,
                               og * DCH * P : (og + 1) * DCH * P],
                        )
                        w1ts.append(wt)
                    fps = [psum.tile([P, TOK], f32, name=f"ffps{j}", tag="ps")
                           for j in range(DCH)]
                    for ic in range(DCH):
                        for j in range(DCH):
                            nc.tensor.matmul(
                                fps[j][:],
                                lhsT=w1ts[ic][:, j * P : (j + 1) * P],
                                rhs=xn1[ic][:],
                                start=(ic == 0),
                                stop=(ic == DCH - 1),
                            )
                    for j in range(DCH):
                        oc = og * DCH + j
                        ft = ffp.tile([P, TOK], f32r, name="fft", tag="ff", bufs=25)
                        nc.vector.tensor_scalar(
                            ft[:], fps[j][:], b1t[:, oc : oc + 1], 0.0,
                            op0=OP.add, op1=OP.max,
                        )
                        ffs.append(ft)
                yps = [psum.tile([P, TOK], f32, name=f"yps{oc}", tag="ps")
                       for oc in range(DCH)]
                for fc in range(FCH):
                    wt = wpool.tile([P, D], f32r, name="w2t", tag="w", bufs=6)
                    nc.sync.dma_start(wt[:], w2[l, fc * P : (fc + 1) * P, :])
                    for oc in range(DCH):
                        nc.tensor.matmul(
                            yps[oc][:],
                            lhsT=wt[:, oc * P : (oc + 1) * P],
                            rhs=ffs[fc][:],
                            start=(fc == 0),
                            stop=(fc == FCH - 1),
                        )
                t2 = []
                for oc in range(DCH):
                    t = acts.tile([P, TOK], f32r, name=f"t2_{oc}", tag="x", bufs=20)
                    nc.vector.tensor_scalar_add(t[:], yps[oc][:], b2t[:, oc : oc + 1])
                    nc.vector.tensor_add(t[:], t[:], xn1[oc][:])
                    t2.append(t)
                x = ln_T(t2, g2t, be2t, f"xl{l}_")

        # ---------------- final LN + AllGather h ----------------
        hT = ln_T(x, gft, bft, "hT_")
        hinr = hin.rearrange("(d p t) -> d p t", p=P, t=TOK)
        for dd in range(DCH):
            nc.sync.dma_start(hinr[dd], hT[dd][:])
        nc.gpsimd.collective_compute(
            "AllGather",
            OP.bypass,
            replica_groups=ALL_GROUP,
            ins=[hin.opt()],
            outs=[hout.opt()],
        )

        with ExitStack() as hctx:
            hpool = hctx.enter_context(tc.tile_pool(name="hpool", bufs=6))
            wopool = hctx.enter_context(tc.tile_pool(name="wopool", bufs=7))
            osb = hctx.enter_context(tc.tile_pool(name="osb", bufs=6))

            HT = [hpool.tile([P, NCORE * TOK], f32r, name=f"HT{dd}", tag=f"h{dd}",
                             bufs=1)
                  for dd in range(DCH)]
            for j in range(NCORE):
                hj = hout[j * D * TOK : (j + 1) * D * TOK].rearrange(
                    "(d p t) -> d p t", p=P, t=TOK
                )
                for dd in range(DCH):
                    nc.sync.dma_start(HT[dd][:, j * TOK : (j + 1) * TOK], hj[dd])

            for vh in range(2):
                wts = []
                for dd in range(DCH):
                    wt = wopool.tile([P, VPAD // 2], f32r, name="woutt",
                                     tag="wo", bufs=7)
                    nc.sync.dma_start(
                        wt[:],
                        woutc[dd * P : (dd + 1) * P,
                              vh * (VPAD // 2) : (vh + 1) * (VPAD // 2)],
                    )
                    wts.append(wt)
                for vc in range(VCH // 2):
                    vch = vh * (VCH // 2) + vc
                    for j in range(4):
                        lp = psum.tile([P, 512], f32, name="logps", tag="ps")
                        for dd in range(DCH):
                            nc.tensor.matmul(
                                lp[:],
                                lhsT=wts[dd][:, vc * P : (vc + 1) * P],
                                rhs=HT[dd][:, j * 512 : (j + 1) * 512],
                                start=(dd == 0),
                                stop=(dd == DCH - 1),
                            )
                        ot = osb.tile([P, 512], f32, name="lsb", tag="lsb", bufs=6)
                        nc.vector.tensor_scalar_add(
                            ot[:], lp[:], boutt[:, vch : vch + 1]
                        )
                        nc.sync.dma_start(
                            out[vch * P : (vch + 1) * P, j * 512 : (j + 1) * 512],
                            ot[:],
                        )

    return nc


_CACHED = {}


def _compiled():
    if "nc" not in _CACHED:
        nc = bacc.Bacc("TRN2", target_bir_lowering=False, debug=False,
                       num_devices=NCORE)
        build(nc)
        nc.compile()
        _CACHED["nc"] = nc
    return _CACHED["nc"]


def _make_inputs(tokens, emb, pe, wq, bq, wk, bk, wv, bv, wo, bo,
                 w1, b1, w2, b2, g1, be1, g2, be2, gf, bf, wout, bout):
    """Build the 8 per-core input maps."""
    f = np.float32
    tokens = np.asarray(tokens).astype(np.int32)

    def parr(b):  # [L, dim] -> [L, P, dim//P]
        b = np.asarray(b, f)
        return np.ascontiguousarray(
            b.reshape(L, b.shape[1] // P, P).transpose(0, 2, 1))

    def parr1(b):  # [dim] -> [P, dim//P]
        b = np.asarray(b, f)
        return np.ascontiguousarray(b.reshape(b.shape[0] // P, P).T)

    common = {
        "emb": np.ascontiguousarray(np.asarray(emb, f)),
        "wq": np.ascontiguousarray(np.asarray(wq, f)),
        "wk": np.ascontiguousarray(np.asarray(wk, f)),
        "wv": np.ascontiguousarray(np.asarray(wv, f)),
        "wo": np.ascontiguousarray(np.asarray(wo, f)),
        "w1": np.ascontiguousarray(np.asarray(w1, f)),
        "w2": np.ascontiguousarray(np.asarray(w2, f)),
        "bqp": parr(bq), "bkp": parr(bk), "bvp": parr(bv),
        "bop": parr(bo), "b1p": parr(b1), "b2p": parr(b2),
        "g1p": parr(g1), "be1p": parr(be1), "g2p": parr(g2), "be2p": parr(be2),
        "gfp": parr1(gf), "bfp": parr1(bf),
    }
    pe = np.asarray(pe, f)
    wout = np.asarray(wout, f)
    bout = np.asarray(bout, f)

    in_maps = []
    for c in range(NCORE):
        b, r = divmod(c, GRP)
        chunks = (r, 7 - r)
        rows = np.concatenate(
            [np.arange(ch * P, (ch + 1) * P) for ch in chunks])
        tok_c = np.stack(
            [tokens[b, ch * P : (ch + 1) * P] for ch in chunks], axis=1
        ).astype(np.int32)  # [P, 2]
        peT_c = np.ascontiguousarray(pe[rows].T)  # [D, TOK]

        # mask [8*P, TOK]: key half-block kb=(g,chi): seq chunk g if chi==0
        # else 7-g; query col qq -> chunk r (qq<128) or 7-r.
        kpos = np.empty(8 * P, np.int64)
        for kb in range(8):
            g, chi = divmod(kb, 2)
            ch = g if chi == 0 else 7 - g
            kpos[kb * P : (kb + 1) * P] = np.arange(ch * P, (ch + 1) * P)
        qpos = rows
        mask = np.where(kpos[:, None] <= qpos[None, :], 0.0, -1e9).astype(f)

        wslice = np.zeros((D, VPAD), f)
        wslice[:, :4000] = wout[:, c * 4000 : (c + 1) * 4000]
        bslice = np.zeros((VPAD,), f)
        bslice[:4000] = bout[c * 4000 : (c + 1) * 4000]
        boutp_c = np.ascontiguousarray(bslice.reshape(VCH, P).T)

        m = dict(common)
        m.update({
            "tok": tok_c,
            "peT": peT_c,
            "maskt": np.ascontiguousarray(mask),
            "woutc": np.ascontiguousarray(wslice),
            "boutp": boutp_c,
        })
        in_maps.append(m)
    return in_maps


def run(in_maps, **kwargs):
    nc = _compiled()
    return run_bass_kernel_spmd(nc, in_maps, list(range(NCORE)), **kwargs)


def assemble(results):
    """results[c]['out'] [VPAD, 8*TOK] -> full logits [B, S, V]."""
    full = np.empty((B, S, V), np.float32)
    for c in range(NCORE):
        lt = np.asarray(results[c]["out"])[:4000]  # [4000, 2048]
        lg = lt.T  # [2048, 4000]
        for j in range(NCORE):
            bj, rj = divmod(j, GRP)
            for hi, ch in enumerate((rj, 7 - rj)):
                full[bj, ch * P : (ch + 1) * P, c * 4000 : (c + 1) * 4000] = \
                    lg[j * TOK + hi * P : j * TOK + (hi + 1) * P]
    return full


def kernel(**inputs):
    in_maps = _make_inputs(**inputs)
    res = run(in_maps)
    return assemble(res.results)
